# revision 14
# baseline (speedup 1.0000x reference)
"""LocalAttention Trainium2 Bass kernel.

Problem: B=8, L=7936, C=256, WINDOW=31.  y = proj(attn(qkv(x))) with
window-local softmax attention (nW=256 independent windows of 31 tokens
per batch).

Sharding: data-parallel over batch — core b processes x[b] ([7936, 256]),
weights replicated.  Output gathered to [8, 7936, 256].

Per-core dataflow (all matmuls fp16 inputs, f32 PSUM accumulation):
  - x [t,c] f32 tiles -> cast fp16 -> PE-transpose -> resident xT [c,t] fp16
  - qT/kT [j,t] = WqkvT-slices.T @ xT   (j on partitions; feature-major)
  - v [t,c]   = xT-slices.T @ WvT       (token-major, needed as AV lhsT)
  - scoresT [k,q] per 124-token block (4 windows): kT.T @ qT, block-diag only
  - attn = exp(scores/16) (ACT, fused scale+cast) * blockdiag mask (DVE)
  - sums via ones-matmul, reciprocal, K=1 broadcast-matmul, normalize attn
  - outT [c,q] = v.T @ attn ; proj [t,j] = outT.T @ WprojT ; +bias ; DMA out
"""

import sys

for _p in ("/opt/trn_rl_repo",):
    if _p not in sys.path:
        sys.path.insert(0, _p)

import numpy as np

import concourse.bass as bass
import concourse.bacc as bacc_mod
import concourse.tile as tile
from concourse import mybir
from concourse.masks import make_identity

F32 = mybir.dt.float32
F16 = mybir.dt.float16

B, L, C = 8, 7936, 256
WS = 31
BLK = 4 * WS          # 124 tokens = 4 windows per attention block
GRP = 4 * BLK         # 496 tokens per outer group
N_GRP = L // GRP      # 16
N_T128 = L // 128     # 62
SCALE = 1.0 / 16.0    # 1/sqrt(C)

_CACHE = {}


def _build_nc():
    nc = bacc_mod.Bacc("TRN2", target_bir_lowering=False, debug=False, num_devices=8)
    x_d = nc.dram_tensor("x", [L, C], F32, kind="ExternalInput").ap()
    wqkv_d = nc.dram_tensor("wqkv", [3 * C, C], F32, kind="ExternalInput").ap()
    bqkv_d = nc.dram_tensor("bqkv", [3 * C], F32, kind="ExternalInput").ap()
    wproj_d = nc.dram_tensor("wproj", [C, C], F32, kind="ExternalInput").ap()
    bproj_d = nc.dram_tensor("bproj", [C], F32, kind="ExternalInput").ap()
    y_d = nc.dram_tensor("y", [L, C], F32, kind="ExternalOutput").ap()

    with tile.TileContext(nc) as tc:
        _emit(tc, x_d, wqkv_d, bqkv_d, wproj_d, bproj_d, y_d)
    nc.compile()
    return nc


def _emit(tc, x_d, wqkv_d, bqkv_d, wproj_d, bproj_d, y_d):
    nc = tc.nc
    from contextlib import ExitStack

    ctx = ExitStack()
    consts = ctx.enter_context(tc.tile_pool(name="consts", bufs=1))
    xt_pool = ctx.enter_context(tc.tile_pool(name="xt", bufs=1))
    xstage = ctx.enter_context(tc.tile_pool(name="xstage", bufs=3))
    qk_pool = ctx.enter_context(tc.tile_pool(name="qk", bufs=2))
    v_pool = ctx.enter_context(tc.tile_pool(name="vp", bufs=2))
    attn_pool = ctx.enter_context(tc.tile_pool(name="attn", bufs=2))
    outt_pool = ctx.enter_context(tc.tile_pool(name="outt", bufs=2))
    y_pool = ctx.enter_context(tc.tile_pool(name="yp", bufs=4))
    small_pool = ctx.enter_context(tc.tile_pool(name="small", bufs=2))

    # PSUM: 8 banks x 2KB.  4 tags x 2 bufs = 8 banks.
    ps_a = ctx.enter_context(tc.tile_pool(name="ps_a", bufs=2, space="PSUM"))
    ps_b = ctx.enter_context(tc.tile_pool(name="ps_b", bufs=2, space="PSUM"))
    ps_c = ctx.enter_context(tc.tile_pool(name="ps_c", bufs=2, space="PSUM"))
    ps_d = ctx.enter_context(tc.tile_pool(name="ps_d", bufs=2, space="PSUM"))

    # ---------------- constants / weights prep ----------------
    ident_h = consts.tile([128, 128], F16)
    make_identity(nc, ident_h[:])
    ident_f = consts.tile([128, 128], F32)
    make_identity(nc, ident_f[:])

    # Wqkv [768,256] -> WqkvT [c_in=128, c_out=2, j=768] fp16
    w_raw = consts.tile([128, 6, C], F32)
    nc.sync.dma_start(w_raw[:], wqkv_d.rearrange("(o p) c -> p o c", p=128))
    w_h = consts.tile([128, 6, C], F16)
    nc.vector.tensor_copy(w_h[:], w_raw[:])
    wqkvT = consts.tile([128, 2, 3 * C], F16)
    for o in range(6):
        for cs in range(2):
            pt = ps_a.tile([128, 128], F16, tag="A", name=f"wtp_{o}_{cs}")
            nc.tensor.transpose(pt[:], w_h[:, o, cs * 128:(cs + 1) * 128], ident_h[:])
            nc.scalar.copy(wqkvT[:, cs, o * 128:(o + 1) * 128], pt[:])

    # Wproj [256,256] -> WprojT [128, 2, 256] fp16
    w2_raw = consts.tile([128, 2, C], F32)
    nc.sync.dma_start(w2_raw[:], wproj_d.rearrange("(o p) c -> p o c", p=128))
    w2_h = consts.tile([128, 2, C], F16)
    nc.vector.tensor_copy(w2_h[:], w2_raw[:])
    wprojT = consts.tile([128, 2, C], F16)
    for o in range(2):
        for cs in range(2):
            pt = ps_a.tile([128, 128], F16, tag="A", name=f"wpt_{o}_{cs}")
            nc.tensor.transpose(pt[:], w2_h[:, o, cs * 128:(cs + 1) * 128], ident_h[:])
            nc.scalar.copy(wprojT[:, cs, o * 128:(o + 1) * 128], pt[:])

    # biases
    bqkv_sb = consts.tile([1, 3 * C], F32)
    nc.sync.dma_start(bqkv_sb[:], bqkv_d[None, :])
    bproj_sb = consts.tile([1, C], F32)
    nc.sync.dma_start(bproj_sb[:], bproj_d[None, :])

    # q/k bias columns [128, 4] (per-partition bias for ACT copies)
    bcol = consts.tile([128, 4], F32)
    for jt in range(4):
        pb = ps_b.tile([128, 4], F32, tag="B", name=f"bcol_{jt}")
        nc.tensor.transpose(
            pb[:, 0:1], bqkv_sb[:1, jt * 128:(jt + 1) * 128], ident_f[:1, :1]
        )
        nc.scalar.copy(bcol[:, jt:jt + 1], pb[:, 0:1])

    ones_col = consts.tile([BLK, 1], F16)
    nc.vector.memset(ones_col[:], 1.0)
    ones_row = consts.tile([1, BLK], F16)
    nc.vector.memset(ones_row[:], 1.0)

    # v / proj broadcast biases [BLK, 256] f32 via K=1 matmul
    bv_h = consts.tile([1, C], F16)
    nc.vector.tensor_copy(bv_h[:], bqkv_sb[:1, 2 * C:3 * C])
    bp_h = consts.tile([1, C], F16)
    nc.vector.tensor_copy(bp_h[:], bproj_sb[:])
    bv_bc = consts.tile([BLK, 2, C], F32)
    bp_bc = consts.tile([BLK, 2, C], F32)
    for src, dst, nm in ((bv_h, bv_bc, "bv"), (bp_h, bp_bc, "bp")):
        pb = ps_b.tile([BLK, C], F32, tag="B", name=f"bbc_{nm}")
        nc.tensor.matmul(pb[:], ones_row[:], src[:], start=True, stop=True)
        nc.scalar.copy(dst[:, 0, :], pb[:])
        nc.scalar.copy(dst[:, 1, :], pb[:])

    # block-diagonal window mask [BLK, GRP] fp16 (pattern repeats per block)
    mask_np = np.zeros((BLK, GRP), dtype=np.float16)
    for j4 in range(4):
        for w in range(4):
            mask_np[w * WS:(w + 1) * WS, j4 * BLK + w * WS: j4 * BLK + (w + 1) * WS] = 1.0
    mask_d = nc.inline_tensor(mask_np, name="maskc")
    mask_sb = consts.tile([BLK, GRP], F16)
    nc.sync.dma_start(mask_sb[:], mask_d.ap())

    # ---------------- phase 1: x -> xT (fp16, resident) ----------------
    # Emitted interleaved with phase 2 (pairs of 128-token tiles; packed
    # transpose PSUM so one DVE copy moves 2 tiles x 2 c-slices).
    xT = xt_pool.tile([128, 2, L + 32], F16)
    nc.gpsimd.memset(xT[:, :, L:L + 32], 0.0)
    N_PAIR = L // 256  # 31

    def emit_pair(p):
        x_f = xstage.tile([128, 2, C], F32, tag="xf", name=f"xf_{p}")
        nc.sync.dma_start(
            x_f[:], x_d[p * 256:(p + 1) * 256, :].rearrange("(a p) c -> p a c", p=128)
        )
        x_h = xstage.tile([128, 2, C], F16, tag="xh", name=f"xh_{p}")
        nc.gpsimd.tensor_copy(x_h[:], x_f[:])
        pt = ps_a.tile([128, 2, 2, 128], F16, tag="A", name=f"xtp_{p}")
        for tt in range(2):
            for cs in range(2):
                nc.tensor.transpose(
                    pt[:, cs, tt, :], x_h[:, tt, cs * 128:(cs + 1) * 128], ident_h[:]
                )
        nc.vector.tensor_copy(
            xT[:, :, p * 256:(p + 1) * 256],
            pt[:].rearrange("p a b c -> p a (b c)"),
        )

    # ---------------- phase 2: per 496-token group ----------------
    pairs_done = 0
    for g in range(N_GRP):
        t0 = g * GRP
        need = min(N_PAIR, (t0 + GRP + 255) // 256)
        while pairs_done < need:
            emit_pair(pairs_done)
            pairs_done += 1
        # qT / kT: [j_inner=128, jt, t] fp16
        qT = qk_pool.tile([128, 2, GRP], F16, tag="qT", name=f"qT_{g}")
        kT = qk_pool.tile([128, 2, 512], F16, tag="kT", name=f"kT_{g}")
        nc.gpsimd.memset(kT[:, :, GRP:512], 0.0)
        for jt in range(4):
            pq = ps_a.tile([128, GRP], F32, tag="A", name=f"pqk_{g}_{jt}")
            for csl in range(2):
                nc.tensor.matmul(
                    pq[:],
                    wqkvT[:, csl, jt * 128:(jt + 1) * 128],
                    xT[:, csl, t0:t0 + GRP],
                    start=(csl == 0),
                    stop=(csl == 1),
                )
            if jt < 2:
                nc.scalar.activation(
                    qT[:, jt, :], pq[:],
                    mybir.ActivationFunctionType.Identity,
                    bias=bcol[:, jt:jt + 1], scale=1.0,
                )
            else:
                nc.vector.tensor_scalar(
                    kT[:, jt - 2, 0:GRP], pq[:], bcol[:, jt:jt + 1], None,
                    mybir.AluOpType.add,
                )

        # v token-major [124, 4, 256] fp16 (+bias); 2 sub-blocks per PSUM
        v_sb = v_pool.tile([BLK, 4, C], F16, tag="v", name=f"v_{g}")
        for pr in range(2):
            pv = ps_b.tile([128, 512], F32, tag="B", name=f"pv_{g}_{pr}")
            for half in range(2):
                t1 = t0 + (2 * pr + half) * BLK
                for csl in range(2):
                    nc.tensor.matmul(
                        pv[:, half * C:(half + 1) * C],
                        xT[:, csl, t1:t1 + 128],
                        wqkvT[:, csl, 2 * C:3 * C],
                        start=(csl == 0),
                        stop=(csl == 1),
                    )
            nc.vector.tensor_add(
                v_sb[:, 2 * pr:2 * pr + 2, :],
                pv[0:BLK, :].rearrange("p (a c) -> p a c", a=2),
                bv_bc[:],
            )

        # scoresT -> exp -> attn [124, 496] fp16 (one psum, one exp)
        attn = attn_pool.tile([BLK, GRP], F16, tag="at", name=f"at_{g}")
        psm = ps_c.tile([128, GRP], F32, tag="C", name=f"ps_{g}")
        for j4 in range(4):
            for csl in range(2):
                nc.tensor.matmul(
                    psm[:, j4 * BLK:(j4 + 1) * BLK],
                    kT[:, csl, j4 * BLK:j4 * BLK + 128],
                    qT[:, csl, j4 * BLK:(j4 + 1) * BLK],
                    start=(csl == 0),
                    stop=(csl == 1),
                )
        nc.scalar.activation(
            attn[:], psm[0:BLK, :],
            mybir.ActivationFunctionType.Exp, scale=SCALE,
        )
        nc.gpsimd.tensor_mul(attn[:], attn[:], mask_sb[:])

        # column sums -> reciprocal -> broadcast -> normalize attn
        psum_s = ps_c.tile([1, GRP], F32, tag="C", name=f"psum_{g}")
        nc.tensor.matmul(psum_s[:], ones_col[:], attn[:], start=True, stop=True)
        recip_f = small_pool.tile([1, GRP], F32, tag="rf", name=f"rf_{g}")
        nc.vector.reciprocal(recip_f[:], psum_s[:])
        recip_h = small_pool.tile([1, GRP], F16, tag="rh", name=f"rh_{g}")
        nc.gpsimd.tensor_copy(recip_h[:], recip_f[:])
        pbc = ps_d.tile([BLK, GRP], F32, tag="D", name=f"pbc_{g}")
        nc.tensor.matmul(pbc[:], ones_row[:], recip_h[:], start=True, stop=True)
        nc.vector.tensor_mul(attn[:], attn[:], pbc[:])

        # AV: outT [c_inner=128, cs, q] fp16
        outT = outt_pool.tile([128, 2, 512], F16, tag="oT", name=f"oT_{g}")
        nc.gpsimd.memset(outT[:, :, GRP:512], 0.0)
        for csl in range(2):
            pav = ps_d.tile([128, GRP], F32, tag="D", name=f"pav_{g}_{csl}")
            for j4 in range(4):
                nc.tensor.matmul(
                    pav[:, j4 * BLK:(j4 + 1) * BLK],
                    v_sb[:, j4, csl * 128:(csl + 1) * 128],
                    attn[:, j4 * BLK:(j4 + 1) * BLK],
                    start=True,
                    stop=True,
                )
            nc.scalar.copy(outT[:, csl, 0:GRP], pav[:])

        # proj + bias -> y; 2 blocks per PSUM (128-wide lhsT for FWL)
        for pr in range(2):
            pp = ps_b.tile([128, 512], F32, tag="B", name=f"pp_{g}_{pr}")
            for half in range(2):
                j4 = 2 * pr + half
                for csl in range(2):
                    nc.tensor.matmul(
                        pp[:, half * C:(half + 1) * C],
                        outT[:, csl, j4 * BLK:j4 * BLK + 128],
                        wprojT[:, csl, :],
                        start=(csl == 0),
                        stop=(csl == 1),
                    )
            y_sb = y_pool.tile([BLK, 2, C], F32, tag="y", name=f"y_{g}_{pr}")
            nc.vector.tensor_add(
                y_sb[:],
                pp[0:BLK, :].rearrange("p (a c) -> p a c", a=2),
                bp_bc[:],
            )
            t1 = t0 + pr * 2 * BLK
            nc.sync.dma_start(
                y_d[t1:t1 + 2 * BLK, :].rearrange("(a p) c -> p a c", p=BLK),
                y_sb[:],
            )

    ctx.close()


def kernel(x, Wqkv, bqkv, Wproj, bproj):
    from concourse.bass_utils import run_bass_kernel_spmd

    if "nc" not in _CACHE:
        _CACHE["nc"] = _build_nc()
    nc = _CACHE["nc"]

    x = np.ascontiguousarray(np.asarray(x, dtype=np.float32))
    wqkv = np.ascontiguousarray(np.asarray(Wqkv, dtype=np.float32))
    bq = np.ascontiguousarray(np.asarray(bqkv, dtype=np.float32))
    wproj = np.ascontiguousarray(np.asarray(Wproj, dtype=np.float32))
    bp = np.ascontiguousarray(np.asarray(bproj, dtype=np.float32))

    in_maps = [
        {"x": x[b], "wqkv": wqkv, "bqkv": bq, "wproj": wproj, "bproj": bp}
        for b in range(B)
    ]
    res = run_bass_kernel_spmd(nc, in_maps, core_ids=list(range(B)))
    return np.stack([r["y"] for r in res.results], axis=0)
